# revision 1
# baseline (speedup 1.0000x reference)
"""Trainium2 Bass kernel for nn_GRF_HGNN_K4 (heterogeneous GNN message passing).

Strategy
--------
* Destination-sharded data parallelism across 8 NeuronCores. Nodes of each
  type are relabeled (degree-sorted, snake-assigned) so every core owns a
  contiguous shard of every node-type table and the per-position tile
  schedule is identical on all cores (SPMD requirement).
* Message aggregation (gather + segment-sum) is done as one-hot matmuls:
  edges sorted by destination are processed in 128-edge tiles; an indirect
  DMA gathers the 128 source rows, a [128e x 256d] selection matrix S
  (S[e,d] = dst_local[e]==d) is built with one tensor_scalar is_equal, and
  TensorE accumulates  aggT[h, d] += msgs.T @ S  in PSUM.
* All per-node transforms run in "transposed" [feature, node] space so the
  GraphConv/MLP weights are stationary lhsT operands, biases are
  per-partition ACT operands, and only two 128x128 PE transposes per
  128-node block are needed (old rows for the root term, result rows for
  the row-major table write).
* After each layer every core AllGathers its new shard into a full
  replicated table (internal Shared DRAM) that next layer's indirect
  gathers read. Layer 3 only produces j (+local f), layer 4 only computes
  foot nodes fused with the decoder.
"""

import numpy as np

import concourse.bass as bass
import concourse.bacc as bacc
import concourse.mybir as mybir
import concourse.tile as tile
from concourse.bass_utils import run_bass_kernel_spmd
from concourse.masks import make_identity

F32 = mybir.dt.float32
I32 = mybir.dt.int32

# ---------------------------------------------------------------- config ----
H = 128
L = 4
PAIR = 256        # one-hot width (dst columns per S tile)
GRP = 512         # transform group width (4 x 128 blocks)
PADLOC = 60000.0  # dst_local pad value (never matches iota 0..255)

# name -> (src type, dst type, mean?, index into conv_* arrays)
ETS = {
    'bj': ('b', 'j', False, 0),
    'jb': ('j', 'b', False, 1),
    'jj': ('j', 'j', False, 2),
    'jf': ('j', 'f', False, 3),
    'fj': ('f', 'j', False, 4),
    'gt': ('b', 'b', True, 5),
    'gs': ('b', 'b', True, 6),
}
DST_ETS = {'j': ('bj', 'jj', 'fj'), 'b': ('jb', 'gt', 'gs'), 'f': ('jf',)}
# node types computed per layer (1-indexed); layer 4 fuses the decoder
LAYER_TYPES = {1: ('j', 'b', 'f'), 2: ('j', 'b', 'f'), 3: ('j', 'f'), 4: ('f',)}
# (layer, type) pairs whose shard is AllGathered into a full table
AG_NEEDED = {(0, 'b'), (0, 'j'), (0, 'f'),
             (1, 'b'), (1, 'j'), (1, 'f'),
             (2, 'b'), (2, 'j'), (2, 'f'),
             (3, 'j')}

FULL_CFG = dict(
    M=8,
    N=dict(b=131072, j=393216, f=131072),
    x_in=dict(b=12, j=3, f=2),
)


# ---------------------------------------------------------- preprocessing ----
def _snake_perm(deg, M, Nc):
    """Return perm (new -> old) so nodes are degree-balanced across cores and
    degree-sorted within each core. new id = core * Nc + rank."""
    order = np.argsort(-deg, kind='stable')          # old ids, degree desc
    n = len(deg)
    pos = np.arange(n)
    cyc = pos % (2 * M)
    core = np.where(cyc < M, cyc, 2 * M - 1 - cyc)   # snake assignment
    # rank within core preserving order
    rank = np.zeros(n, np.int64)
    for c in range(M):
        m = core == c
        rank[m] = np.arange(m.sum())
    new_of_sorted = core * Nc + rank
    perm = np.empty(n, np.int64)                     # new -> old
    perm[new_of_sorted] = order
    old2new = np.empty(n, np.int64)
    old2new[order] = new_of_sorted
    return perm, old2new


def preprocess(inputs, cfg):
    """Compute permutations, padded per-core edge schedules and packed
    weights. Returns a dict consumed by build_program/make_in_maps."""
    M = cfg['M']
    N = cfg['N']
    NC = {t: N[t] // M for t in N}
    for t in N:
        assert N[t] % (M * GRP) == 0, (t, N[t])

    ei = {n: np.asarray(inputs[f'ei_{n}']).astype(np.int64) for n in ETS}

    # in-degree per node type (sum over relevant edge types) for balancing
    deg = {t: np.zeros(N[t], np.int64) for t in 'bjf'}
    for n, (s, d, _, _) in ETS.items():
        deg[d] += np.bincount(ei[n][1], minlength=N[d])

    perm, old2new = {}, {}
    for t in 'bjf':
        perm[t], old2new[t] = _snake_perm(deg[t], M, NC[t])

    # --- per-destination-type tile schedules (shared across layers) ---
    sched = {}
    arrs = {}          # per node type: dict of per-core arrays
    for t in 'bjf':
        npairs = NC[t] // PAIR
        ngroups = NC[t] // GRP
        ets = DST_ETS[t]

        # count edges per (core, pair) per edge type
        edata = {}
        T = {}
        for etn in ets:
            s_t, d_t, mean, _ = ETS[etn]
            sn = old2new[s_t][ei[etn][0]]
            dn = old2new[d_t][ei[etn][1]]
            c = dn // NC[t]
            pl = (dn % NC[t]) // PAIR
            key = c * npairs + pl
            cnts = np.bincount(key, minlength=M * npairs).reshape(M, npairs)
            Tp = -(-cnts.max(0) // 128)               # ceil over max core
            # group fix: if one pair of a group has tiles, both need >= 1
            T2 = Tp.reshape(ngroups, 2)
            gm = T2.max(1)
            T2 = np.where((gm > 0)[:, None] & (T2 == 0), 1, T2)
            T[etn] = T2.reshape(-1)
            w = None
            if mean:
                cnt_d = np.bincount(dn, minlength=N[t]).astype(np.float32)
                w = 1.0 / np.maximum(cnt_d, 1.0)
                w = w[dn]                              # per-edge weight
            edata[etn] = (sn, dn, c, pl, key, w)

        # canonical tile stream: g -> et -> pair(2g, 2g+1)
        tile_base = {}
        tt = 0
        for g in range(ngroups):
            for etn in ets:
                for pr in (2 * g, 2 * g + 1):
                    tile_base[(etn, pr)] = tt
                    tt += int(T[etn][pr])
        Ttot = tt

        idx = np.zeros((M, 128, max(Ttot, 1)), np.int32)
        dl = np.full((M, 128, max(Ttot, 1)), PADLOC, np.float32)
        wts = np.ones((M, 128, max(Ttot, 1)), np.float32) if t == 'b' else None

        for etn in ets:
            sn, dn, c, pl, key, w = edata[etn]
            order = np.argsort(key, kind='stable')
            ks = key[order]
            cnts = np.bincount(ks, minlength=M * npairs)
            starts = np.concatenate([[0], np.cumsum(cnts)[:-1]])
            rank = np.arange(len(ks)) - starts[ks]
            slot = (rank % 128).astype(np.int64)
            k = rank // 128
            bases = np.array([tile_base[(etn, p)] for p in range(npairs)], np.int64)
            tcol = bases[ks % npairs] + k
            cs = c[order]
            idx[cs, slot, tcol] = sn[order].astype(np.int32)
            dl[cs, slot, tcol] = ((dn[order] % NC[t]) % PAIR).astype(np.float32)
            if w is not None:
                wts[cs, slot, tcol] = w[order]

        sched[t] = dict(ngroups=ngroups, T=T, tile_base=tile_base, Ttot=Ttot)
        arrs[t] = dict(idx=idx, dl=dl, wts=wts)

    # --- packed weights (all transposed for lhsT use) and bias columns ---
    wslots, wkeys = [], {}

    def wslot(key, mat):
        m = np.zeros((128, 128), np.float32)
        m[:mat.shape[0], :mat.shape[1]] = mat
        wkeys[key] = len(wslots)
        wslots.append(m)

    TN = {'b': 'base', 'j': 'joint', 'f': 'foot'}
    for t in 'bjf':
        wslot(('enc', t), np.asarray(inputs[f'enc_W_{TN[t]}']).T)
    Wrel = np.asarray(inputs['conv_Wrel'])
    brel = np.asarray(inputs['conv_brel'])
    Wroot = np.asarray(inputs['conv_Wroot'])
    for l in range(L):
        for etn, (_, _, _, ei_idx) in ETS.items():
            wslot(('rel', l, etn), Wrel[l, ei_idx].T)
        for t in 'bjf':
            root = sum(Wroot[l, ETS[etn][3]] for etn in DST_ETS[t])
            wslot(('root', l, t), np.asarray(root).T)
    wslot(('mlp1',), np.asarray(inputs['bt_W1']).T)
    wslot(('mlp2',), np.asarray(inputs['bt_W2']).T)
    wslot(('dec',), np.asarray(inputs['dec_W']).T)        # [H, 1]
    WPK = np.stack(wslots)                                 # [nw, 128, 128]

    bcols, bkeys = [], {}

    def bcol(key, v):
        col = np.zeros(128, np.float32)
        col[:len(v)] = v
        bkeys[key] = len(bcols)
        bcols.append(col)

    for t, nm in (('b', 'base'), ('j', 'joint'), ('f', 'foot')):
        bcol(('enc', t), np.asarray(inputs[f'enc_b_{nm}']))
    for l in range(L):
        for t in 'bjf':
            bcol(('agg', l, t), sum(brel[l, ETS[etn][3]] for etn in DST_ETS[t]))
    bcol(('mlp1',), np.asarray(inputs['bt_b1']))
    bcol(('mlp2',), np.asarray(inputs['bt_b2']))
    BPK = np.stack(bcols, axis=1)                          # [128, nb]

    iota = np.tile(np.arange(PAIR, dtype=np.float32), (128, 1))  # [128, 256]

    # --- per-core transposed, permuted input features ---
    xT = {}
    for t, nm in (('b', 'base'), ('j', 'joint'), ('f', 'foot')):
        xp = np.asarray(inputs[f'x_{nm}'])[perm[t]]        # [N, in] permuted
        xT[t] = np.ascontiguousarray(
            xp.reshape(M, NC[t], -1).transpose(0, 2, 1)).astype(np.float32)

    return dict(cfg=cfg, NC=NC, perm=perm, old2new=old2new, sched=sched,
                arrs=arrs, WPK=WPK, wkeys=wkeys, BPK=BPK, bkeys=bkeys,
                iota=iota, xT=xT, dec_b=float(np.asarray(inputs['dec_b'])[0]))


def make_in_maps(prep):
    M = prep['cfg']['M']
    maps = []
    for c in range(M):
        m = dict(
            wpk=prep['WPK'], bpk=prep['BPK'], iota=prep['iota'],
            xt_b=prep['xT']['b'][c], xt_j=prep['xT']['j'][c],
            xt_f=prep['xT']['f'][c],
        )
        for t in 'bjf':
            m[f'idx_{t}'] = prep['arrs'][t]['idx'][c]
            m[f'dl_{t}'] = prep['arrs'][t]['dl'][c]
        m['wts_b'] = prep['arrs']['b']['wts'][c]
        maps.append(m)
    return maps


# ------------------------------------------------------------ bass program ----
def build_program(prep):
    cfg = prep['cfg']
    M = cfg['M']
    NC = prep['NC']
    N = cfg['N']
    x_in = cfg['x_in']
    sched = prep['sched']
    wkeys, bkeys = prep['wkeys'], prep['bkeys']

    nc = bacc.Bacc('TRN2', target_bir_lowering=False, num_devices=M)

    # ---- I/O ----
    P = {}
    P['wpk'] = nc.declare_dram_parameter('wpk', list(prep['WPK'].shape), F32, isOutput=False)
    P['bpk'] = nc.declare_dram_parameter('bpk', list(prep['BPK'].shape), F32, isOutput=False)
    P['iota'] = nc.declare_dram_parameter('iota', [128, PAIR], F32, isOutput=False)
    for t in 'bjf':
        P[f'xt_{t}'] = nc.declare_dram_parameter(f'xt_{t}', [x_in[t], NC[t]], F32, isOutput=False)
        ncols = max(sched[t]['Ttot'], 1)
        P[f'idx_{t}'] = nc.declare_dram_parameter(f'idx_{t}', [128, ncols], I32, isOutput=False)
        P[f'dl_{t}'] = nc.declare_dram_parameter(f'dl_{t}', [128, ncols], F32, isOutput=False)
    P['wts_b'] = nc.declare_dram_parameter('wts_b', [128, max(sched['b']['Ttot'], 1)], F32, isOutput=False)
    out_dec = nc.declare_dram_parameter('out', [NC['f'], 1], F32, isOutput=True)

    # ---- internal DRAM ----
    S = {}   # shard tables written by this core
    HT = {}  # allgathered full tables
    for l in range(0, L):
        for t in 'bjf':
            if l == 0 or t in LAYER_TYPES[l]:
                if l == 3 and t == 'b':
                    continue
                S[(l, t)] = nc.dram_tensor(f's{l}{t}', [NC[t], H], F32)
    for (l, t) in AG_NEEDED:
        HT[(l, t)] = nc.dram_tensor(f'h{l}{t}', [N[t], H], F32, addr_space='Shared')

    RG = [list(range(M))]

    with tile.TileContext(nc) as tc:
        cpool = tc.alloc_tile_pool(name='consts', bufs=1)
        idxpool = tc.alloc_tile_pool(name='idxres', bufs=1)
        xpool = tc.alloc_tile_pool(name='xin', bufs=2)
        mpool = tc.alloc_tile_pool(name='msgs', bufs=12)
        spool = tc.alloc_tile_pool(name='sel', bufs=8)
        apool = tc.alloc_tile_pool(name='aggsb', bufs=4)
        opool = tc.alloc_tile_pool(name='oldsb', bufs=3)
        rpool = tc.alloc_tile_pool(name='ressb', bufs=3)
        agg_ps = tc.alloc_tile_pool(name='aggps', bufs=3, space='PSUM')
        out_ps = tc.alloc_tile_pool(name='outps', bufs=3, space='PSUM')
        tr_ps = tc.alloc_tile_pool(name='trps', bufs=2, space='PSUM')

        # ---- constants ----
        ident = cpool.tile([128, 128], F32, tag='ident', name='ident')
        make_identity(nc, ident[:, :])
        iota_sb = cpool.tile([128, PAIR], F32, tag='iota', name='iota')
        nc.sync.dma_start(out=iota_sb[:, :], in_=P['iota'][:, :])

        nw = prep['WPK'].shape[0]
        wsb = cpool.tile([128, nw * 128], F32, tag='wsb', name='wsb')
        nc.sync.dma_start(out=wsb[:, :], in_=P['wpk'][:, :, :].rearrange('w p h -> p w h'))

        def W(key):
            i = wkeys[key]
            return wsb[:, i * 128:(i + 1) * 128]

        nb = prep['BPK'].shape[1]
        bsb = cpool.tile([128, nb], F32, tag='bsb', name='bsb')
        nc.sync.dma_start(out=bsb[:, :], in_=P['bpk'][:, :])

        def B(key):
            i = bkeys[key]
            return bsb[:, i:i + 1]

        idx_sb, dl_sb, wts_sb = {}, {}, {}
        for t in 'bjf':
            ncols = max(sched[t]['Ttot'], 1)
            idx_sb[t] = idxpool.tile([128, ncols], I32, tag=f'idx{t}', name=f'idx{t}')
            nc.sync.dma_start(out=idx_sb[t][:, :], in_=P[f'idx_{t}'][:, :])
            dl_sb[t] = idxpool.tile([128, ncols], F32, tag=f'dl{t}', name=f'dl{t}')
            nc.sync.dma_start(out=dl_sb[t][:, :], in_=P[f'dl_{t}'][:, :])
        wts_sb['b'] = idxpool.tile([128, max(sched['b']['Ttot'], 1)], F32, tag='wtsb', name='wtsb')
        nc.sync.dma_start(out=wts_sb['b'][:, :], in_=P['wts_b'][:, :])

        # ---- encoder: shard tables S[(0, t)] ----
        for t in 'bjf':
            ngroups = NC[t] // GRP
            for g in range(ngroups):
                xt = xpool.tile([128, GRP], F32, tag='xt', name='xt')
                nc.sync.dma_start(out=xt[:x_in[t], :],
                                  in_=P[f'xt_{t}'][:, g * GRP:(g + 1) * GRP])
                ps = out_ps.tile([128, GRP], F32, tag='outp', name='outp')
                nc.tensor.matmul(out=ps[:, :], lhsT=W(('enc', t))[:x_in[t], :],
                                 rhs=xt[:x_in[t], :], start=True, stop=True)
                s = rpool.tile([128, GRP], F32, tag='res', name='res')
                nc.scalar.activation(s[:, :], ps[:, :],
                                     mybir.ActivationFunctionType.Relu,
                                     bias=B(('enc', t)), scale=1.0)
                tp = tr_ps.tile([128, GRP], F32, tag='trp', name='trp')
                for k in range(GRP // 128):
                    nc.tensor.transpose(out=tp[:, k * 128:(k + 1) * 128],
                                        in_=s[:, k * 128:(k + 1) * 128],
                                        identity=ident[:, :])
                orow = rpool.tile([128, GRP], F32, tag='orow', name='orow')
                nc.any.tensor_copy(orow[:, :], tp[:, :])
                nc.sync.dma_start(
                    out=S[(0, t)][g * GRP:(g + 1) * GRP, :].rearrange('(k p) h -> p k h', p=128),
                    in_=orow[:, :])
            nc.gpsimd.collective_compute(
                'AllGather', mybir.AluOpType.bypass, replica_groups=RG,
                ins=[S[(0, t)][:, :]], outs=[HT[(0, t)][:, :]])

        # ---- layers ----
        for l1 in range(1, L + 1):
            lw = l1 - 1       # weight index
            for t in LAYER_TYPES[l1]:
                ngroups = NC[t] // GRP
                ets = DST_ETS[t]
                sc = sched[t]
                for g in range(ngroups):
                    # old rows (local shard of previous layer)
                    old = opool.tile([128, GRP], F32, tag='old', name='old')
                    nc.sync.dma_start(
                        out=old[:, :],
                        in_=S[(l1 - 1, t)][g * GRP:(g + 1) * GRP, :].rearrange('(k p) h -> p k h', p=128))
                    tp = tr_ps.tile([128, GRP], F32, tag='trp', name='trp')
                    for k in range(GRP // 128):
                        nc.tensor.transpose(out=tp[:, k * 128:(k + 1) * 128],
                                            in_=old[:, k * 128:(k + 1) * 128],
                                            identity=ident[:, :])
                    oldT = opool.tile([128, GRP], F32, tag='oldT', name='oldT')
                    nc.any.tensor_copy(oldT[:, :], tp[:, :])

                    # aggregate each edge type into aggT sbuf tiles
                    aggs = []
                    for etn in ets:
                        T0 = int(sc['T'][etn][2 * g])
                        T1 = int(sc['T'][etn][2 * g + 1])
                        if T0 + T1 == 0:
                            continue
                        t0 = sc['tile_base'][(etn, 2 * g)]
                        src_t = ETS[etn][0]
                        mean = ETS[etn][2]
                        aps = agg_ps.tile([128, GRP], F32, tag='aggp', name='aggp')
                        for pr01, Tn in ((0, T0), (1, T1)):
                            for kk in range(Tn):
                                ti = t0 + pr01 * T0 + kk
                                msgs = mpool.tile([128, 128], F32, tag='msgs', name='msgs')
                                nc.gpsimd.indirect_dma_start(
                                    out=msgs[:, :],
                                    out_offset=None,
                                    in_=HT[(l1 - 1, src_t)][:, :],
                                    in_offset=bass.IndirectOffsetOnAxis(
                                        ap=idx_sb[t][:, ti:ti + 1], axis=0),
                                )
                                sel = spool.tile([128, PAIR], F32, tag='sel', name='sel')
                                nc.any.tensor_scalar(
                                    out=sel[:, :], in0=iota_sb[:, :],
                                    scalar1=dl_sb[t][:, ti:ti + 1], scalar2=None,
                                    op0=mybir.AluOpType.is_equal)
                                if mean:
                                    nc.any.tensor_scalar(
                                        out=msgs[:, :], in0=msgs[:, :],
                                        scalar1=wts_sb['b'][:, ti:ti + 1], scalar2=None,
                                        op0=mybir.AluOpType.mult)
                                nc.tensor.matmul(
                                    out=aps[:, pr01 * PAIR:(pr01 + 1) * PAIR],
                                    lhsT=msgs[:, :], rhs=sel[:, :],
                                    start=(kk == 0), stop=(kk == Tn - 1))
                        asb = apool.tile([128, GRP], F32, tag='aggsb', name='aggsb')
                        nc.any.tensor_copy(asb[:, :], aps[:, :])
                        aggs.append((etn, asb))

                    # transform: out_ps[o, d] = root + sum_et Wrel_et @ aggT_et
                    ops = out_ps.tile([128, GRP], F32, tag='outp', name='outp')
                    nc.tensor.matmul(out=ops[:, :], lhsT=W(('root', lw, t)),
                                     rhs=oldT[:, :], start=True, stop=(not aggs))
                    for i, (etn, asb) in enumerate(aggs):
                        nc.tensor.matmul(out=ops[:, :], lhsT=W(('rel', lw, etn)),
                                         rhs=asb[:, :], start=False,
                                         stop=(i == len(aggs) - 1))

                    if t == 'b':
                        s0 = rpool.tile([128, GRP], F32, tag='res', name='res')
                        nc.any.tensor_scalar(out=s0[:, :], in0=ops[:, :],
                                             scalar1=B(('agg', lw, 'b')), scalar2=None,
                                             op0=mybir.AluOpType.add)
                        ps2 = out_ps.tile([128, GRP], F32, tag='outp', name='outp')
                        nc.tensor.matmul(out=ps2[:, :], lhsT=W(('mlp1',)),
                                         rhs=s0[:, :], start=True, stop=True)
                        t1 = rpool.tile([128, GRP], F32, tag='res', name='res')
                        nc.scalar.activation(t1[:, :], ps2[:, :],
                                             mybir.ActivationFunctionType.Relu,
                                             bias=B(('mlp1',)), scale=1.0)
                        ps3 = out_ps.tile([128, GRP], F32, tag='outp', name='outp')
                        nc.tensor.matmul(out=ps3[:, :], lhsT=W(('mlp2',)),
                                         rhs=t1[:, :], start=True, stop=True)
                        s = rpool.tile([128, GRP], F32, tag='res', name='res')
                        nc.any.tensor_scalar(out=s[:, :], in0=ps3[:, :],
                                             scalar1=B(('mlp2',)), scalar2=None,
                                             op0=mybir.AluOpType.add)
                    else:
                        s = rpool.tile([128, GRP], F32, tag='res', name='res')
                        nc.scalar.activation(s[:, :], ops[:, :],
                                             mybir.ActivationFunctionType.Relu,
                                             bias=B(('agg', lw, t)), scale=1.0)

                    if l1 == L:
                        # fused decoder on foot nodes: hfT = s + oldT; out = hfT.T @ decW + b
                        hfT = rpool.tile([128, GRP], F32, tag='hfT', name='hfT')
                        nc.any.tensor_add(hfT[:, :], s[:, :], oldT[:, :])
                        dps = tr_ps.tile([128, GRP], F32, tag='trp', name='decp')
                        for k in range(GRP // 128):
                            nc.tensor.matmul(out=dps[:, k:k + 1],
                                             lhsT=hfT[:, k * 128:(k + 1) * 128],
                                             rhs=W(('dec',))[:, 0:1],
                                             start=True, stop=True)
                        dsb = rpool.tile([128, GRP // 128], F32, tag='dsb', name='dsb')
                        nc.scalar.activation(dsb[:, :], dps[:, :GRP // 128],
                                             mybir.ActivationFunctionType.Copy,
                                             bias=prep['dec_b'], scale=1.0)
                        nc.sync.dma_start(
                            out=out_dec[g * GRP:(g + 1) * GRP, :].rearrange('(k p) o -> p k o', p=128),
                            in_=dsb[:, :])
                    else:
                        tp2 = tr_ps.tile([128, GRP], F32, tag='trp', name='trp')
                        for k in range(GRP // 128):
                            nc.tensor.transpose(out=tp2[:, k * 128:(k + 1) * 128],
                                                in_=s[:, k * 128:(k + 1) * 128],
                                                identity=ident[:, :])
                        orow = rpool.tile([128, GRP], F32, tag='orow', name='orow')
                        nc.any.tensor_add(orow[:, :], tp2[:, :], old[:, :])
                        nc.sync.dma_start(
                            out=S[(l1, t)][g * GRP:(g + 1) * GRP, :].rearrange('(k p) h -> p k h', p=128),
                            in_=orow[:, :])

                if (l1, t) in AG_NEEDED:
                    nc.gpsimd.collective_compute(
                        'AllGather', mybir.AluOpType.bypass, replica_groups=RG,
                        ins=[S[(l1, t)][:, :]], outs=[HT[(l1, t)][:, :]])

        for p in (tr_ps, out_ps, agg_ps, rpool, opool, apool, spool, mpool,
                  xpool, idxpool, cpool):
            p.release()

    nc.compile()
    return nc


# ------------------------------------------------------------------ driver ----
def _run(inputs, cfg, runner=None):
    prep = preprocess(inputs, cfg)
    in_maps = make_in_maps(prep)
    nc = build_program(prep)
    if runner is None:
        res = run_bass_kernel_spmd(nc, in_maps, list(range(cfg['M']))).results
    else:
        res = runner(nc, in_maps)
    M = cfg['M']
    NF = cfg['N']['f']
    outp = np.concatenate([np.asarray(res[c]['out']) for c in range(M)], 0)  # permuted [NF,1]
    out = np.empty((NF, 1), np.float32)
    out[prep['perm']['f']] = outp
    return out


def kernel(**inputs):
    return _run(inputs, FULL_CFG)



# revision 3
# speedup vs baseline: 10.2491x; 10.2491x over previous
"""Trainium2 Bass kernel for nn_GRF_HGNN_K4 — v2.

Key differences from the v1 baseline:
* fp16 node tables / messages / selection matrices / weights (PSUM stays
  f32): 4x TensorE throughput vs f32 and half the HBM + collective bytes.
* One-hot windows widened to 512 (= the transform group) and nodes are
  re-assigned to (core, window) bins with a per-window degree snake plus
  ~3% virtual-node slack, cutting the padded edge-tile count.
* Replicated encoder: every core encodes the full node tables from the
  (replicated) raw features, eliminating the three layer-0 AllGathers and
  the startup serialization they caused.
* Mean-edge weights are folded into the selection matrix build
  (tensor_scalar is_equal . mult), removing the per-tile message multiply.
* Tables use a partition-interleaved row order (row = 4*p + k within each
  512-node window) so a group's 512 "old" rows are one DMA with 1KB/partition
  lines; layer-1 old rows come from the replicated encoder table via a
  single indirect gather per group (per-core base offsets shipped as data).
* Explicit engine placement: Pool(GpSimd) runs ONLY the indirect gathers
  (~1us software descriptor-gen each — the critical resource) plus the
  collective triggers; HWDGE handles all regular DMA, DVE builds sel and
  residual adds, ACT does PSUM->SBUF casts and activations.
"""

import numpy as np

import concourse.bass as bass
import concourse.bacc as bacc
import concourse.mybir as mybir
import concourse.tile as tile
from concourse.bass_utils import run_bass_kernel_spmd
from concourse.masks import make_identity

F32 = mybir.dt.float32
F16 = mybir.dt.float16
I32 = mybir.dt.int32

H = 128
L = 4
W = 512            # one-hot window == transform group width
M = 8
PADLOC = 60000.0   # fp16-exact, never matches iota 0..511
REPEAT = 1         # whole-kernel repetitions (measurement amplification)

ETS = {
    'bj': ('b', 'j', False, 0),
    'jb': ('j', 'b', False, 1),
    'jj': ('j', 'j', False, 2),
    'jf': ('j', 'f', False, 3),
    'fj': ('f', 'j', False, 4),
    'gt': ('b', 'b', True, 5),
    'gs': ('b', 'b', True, 6),
}
DST_ETS = {'j': ('bj', 'jj', 'fj'), 'b': ('jb', 'gt', 'gs'), 'f': ('jf',)}
LAYER_TYPES = {1: ('j', 'b', 'f'), 2: ('j', 'b', 'f'), 3: ('j', 'f'), 4: ('f',)}
AG_NEEDED = {(1, 'b'), (1, 'j'), (1, 'f'),
             (2, 'b'), (2, 'j'), (2, 'f'),
             (3, 'j')}

FULL_CFG = dict(
    M=M,
    N=dict(b=131072, j=393216, f=131072),
    NCp=dict(b=16896, j=50688, f=16896),   # per-core padded shard (3% virtual)
    x_in=dict(b=12, j=3, f=2),
)


# ---------------------------------------------------------- preprocessing ----
def _interleave(s):
    """window slot s (= k*128 + p) -> storage offset 4*p + k."""
    return 4 * (s % 128) + s // 128


def _window_snake(total_deg, M_, NCp_):
    """Assign nodes (degree-sorted) to (core, window, slot) with a snake
    within every window chunk, padding with virtual slots at the end.
    Returns n_log_of_old (old -> c*NCp + w*512 + s)."""
    n = len(total_deg)
    order = np.argsort(-total_deg, kind='stable')
    slots = M_ * NCp_
    nw = NCp_ // W
    # position i in the padded order -> (chunk w, pos-in-chunk)
    # snake over 2M so consecutive (similar-degree) nodes spread over cores
    i = np.arange(slots)
    w = i // (M_ * W)
    j = i % (M_ * W)
    cyc = j % (2 * M_)
    core = np.where(cyc < M_, cyc, 2 * M_ - 1 - cyc)
    s = (j // (2 * M_)) * 2 + (cyc >= M_)    # slot within window per core
    n_log = core * NCp_ + w * W + s
    n_log_of_old = np.full(n, -1, np.int64)
    n_log_of_old[order] = n_log[:n]
    return n_log_of_old


def preprocess(inputs, cfg):
    M_ = cfg['M']
    N = cfg['N']
    NCp = cfg['NCp']

    ei = {n: np.asarray(inputs[f'ei_{n}']).astype(np.int64) for n in ETS}

    # dst-side degree per node type
    deg = {t: np.zeros(N[t], np.int64) for t in 'bjf'}
    for n, (s_t, d_t, _, _) in ETS.items():
        deg[d_t] += np.bincount(ei[n][1], minlength=N[d_t])

    n_log_of_old, row_of_old = {}, {}
    for t in 'bjf':
        nl = _window_snake(deg[t], M_, NCp[t])
        n_log_of_old[t] = nl
        s = nl % W
        row_of_old[t] = (nl - s) + _interleave(s)

    # --- edge tile schedules per dst type ---
    sched, arrs = {}, {}
    for t in 'bjf':
        nw = NCp[t] // W
        ets = DST_ETS[t]
        T = {}
        edata = {}
        for etn in ets:
            s_t, d_t, mean, _ = ETS[etn]
            sn = row_of_old[s_t][ei[etn][0]]
            dn = n_log_of_old[t][ei[etn][1]]
            c = dn // NCp[t]
            rem = dn % NCp[t]
            wloc = rem // W
            dl = (rem % W).astype(np.float64)
            key = c * nw + wloc
            cnts = np.bincount(key, minlength=M_ * nw).reshape(M_, nw)
            T[etn] = -(-cnts.max(0) // 128)
            wv = None
            if mean:
                cnt_d = np.bincount(ei[etn][1], minlength=N[t]).astype(np.float32)
                wv = (1.0 / np.maximum(cnt_d, 1.0))[ei[etn][1]]
            edata[etn] = (sn, dl, key, wv)

        tile_base = {}
        tt = 0
        for wv_ in range(nw):
            for etn in ets:
                tile_base[(etn, wv_)] = tt
                tt += int(T[etn][wv_])
        Ttot = max(tt, 1)

        idx = np.zeros((M_, 128, Ttot), np.int32)
        dla = np.full((M_, 128, Ttot), PADLOC, np.float32)
        wts = np.ones((M_, 128, Ttot), np.float32) if t == 'b' else None

        for etn in ets:
            sn, dl, key, wv = edata[etn]
            order = np.argsort(key, kind='stable')
            ks = key[order]
            cnts = np.bincount(ks, minlength=M_ * (NCp[t] // W))
            starts = np.concatenate([[0], np.cumsum(cnts)[:-1]])
            rank = np.arange(len(ks)) - starts[ks]
            slot = (rank % 128).astype(np.int64)
            kk = rank // 128
            bases = np.array([tile_base[(etn, p)] for p in range(NCp[t] // W)],
                             np.int64)
            tcol = bases[ks % (NCp[t] // W)] + kk
            cs = ks // (NCp[t] // W)
            idx[cs, slot, tcol] = sn[order].astype(np.int32)
            dla[cs, slot, tcol] = dl[order].astype(np.float32)
            if wv is not None:
                wts[cs, slot, tcol] = wv[order].astype(np.float32)

        sched[t] = dict(nw=NCp[t] // W, T=T, tile_base=tile_base, Ttot=Ttot)
        arrs[t] = dict(idx=idx, dl=dla, wts=wts)

    # --- old-row gather bases for layer 1 (per-core data) ---
    oldidx = {}
    for t in 'bjf':
        ng = NCp[t] // W
        c = np.arange(M_)[:, None, None]
        p = np.arange(128)[None, :, None]
        g = np.arange(ng)[None, None, :]
        oldidx[t] = (c * NCp[t] + g * W + 4 * p).astype(np.int32)

    # --- packed weights (transposed lhsT slots, fp16) ---
    wslots, wkeys = [], {}

    def wslot(key, mat):
        m = np.zeros((128, 128), np.float16)
        m[:mat.shape[0], :mat.shape[1]] = np.asarray(mat, np.float32)
        wkeys[key] = len(wslots)
        wslots.append(m)

    TN = {'b': 'base', 'j': 'joint', 'f': 'foot'}
    for t in 'bjf':
        wt = np.asarray(inputs[f'enc_W_{TN[t]}']).T
        wb = np.asarray(inputs[f'enc_b_{TN[t]}'])[None, :]
        wslot(('enc', t), np.concatenate([wt, wb], axis=0))
    Wrel = np.asarray(inputs['conv_Wrel'])
    brel = np.asarray(inputs['conv_brel'])
    Wroot = np.asarray(inputs['conv_Wroot'])
    for l in range(L):
        for etn, (_, _, _, ei_idx) in ETS.items():
            wslot(('rel', l, etn), Wrel[l, ei_idx].T)
        for t in 'bjf':
            root = sum(Wroot[l, ETS[etn][3]] for etn in DST_ETS[t])
            wslot(('root', l, t), np.asarray(root).T)
    wslot(('mlp1',), np.asarray(inputs['bt_W1']).T)
    wslot(('mlp2',), np.asarray(inputs['bt_W2']).T)
    wslot(('dec',), np.asarray(inputs['dec_W']).T)
    WPK = np.stack(wslots)

    bcols, bkeys = [], {}

    def bcol(key, v):
        col = np.zeros(128, np.float32)
        col[:len(v)] = v
        bkeys[key] = len(bcols)
        bcols.append(col)

    for t, nm in (('b', 'base'), ('j', 'joint'), ('f', 'foot')):
        bcol(('enc', t), np.asarray(inputs[f'enc_b_{nm}']))
    for l in range(L):
        for t in 'bjf':
            bcol(('agg', l, t), sum(brel[l, ETS[etn][3]] for etn in DST_ETS[t]))
    bcol(('mlp1',), np.asarray(inputs['bt_b1']))
    bcol(('mlp2',), np.asarray(inputs['bt_b2']))
    BPK = np.stack(bcols, axis=1)

    iota = np.tile(np.arange(W, dtype=np.float16), (128, 1))

    # --- replicated, transposed, logically-ordered input features ---
    xt = {}
    for t, nm in (('b', 'base'), ('j', 'joint'), ('f', 'foot')):
        x = np.asarray(inputs[f'x_{nm}'], np.float32)
        full = np.zeros((M_ * NCp[t], x.shape[1] + 1), np.float16)
        full[n_log_of_old[t], :-1] = x.astype(np.float16)
        full[:, -1] = 1.0
        xt[t] = np.ascontiguousarray(full.T)

    return dict(cfg=cfg, sched=sched, arrs=arrs, oldidx=oldidx,
                WPK=WPK, wkeys=wkeys, BPK=BPK, bkeys=bkeys, iota=iota, xt=xt,
                n_log_of_old=n_log_of_old,
                dec_b=float(np.asarray(inputs['dec_b'])[0]))


def make_in_maps(prep):
    M_ = prep['cfg']['M']
    maps = []
    for c in range(M_):
        m = dict(wpk=prep['WPK'], bpk=prep['BPK'], iota=prep['iota'],
                 xt_b=prep['xt']['b'], xt_j=prep['xt']['j'],
                 xt_f=prep['xt']['f'])
        for t in 'bjf':
            m[f'idx_{t}'] = prep['arrs'][t]['idx'][c]
            m[f'dl_{t}'] = prep['arrs'][t]['dl'][c]
            m[f'oldidx_{t}'] = prep['oldidx'][t][c]
        m['wts_b'] = prep['arrs']['b']['wts'][c]
        maps.append(m)
    return maps


# ------------------------------------------------------------ bass program ----
def build_program(prep):
    cfg = prep['cfg']
    M_ = cfg['M']
    NCp = cfg['NCp']
    x_in = cfg['x_in']
    sched = prep['sched']
    wkeys, bkeys = prep['wkeys'], prep['bkeys']

    nc = bacc.Bacc('TRN2', target_bir_lowering=False, num_devices=M_)

    P = {}
    P['wpk'] = nc.declare_dram_parameter('wpk', list(prep['WPK'].shape), F16, isOutput=False)
    P['bpk'] = nc.declare_dram_parameter('bpk', list(prep['BPK'].shape), F32, isOutput=False)
    P['iota'] = nc.declare_dram_parameter('iota', [128, W], F16, isOutput=False)
    for t in 'bjf':
        P[f'xt_{t}'] = nc.declare_dram_parameter(f'xt_{t}', [x_in[t] + 1, M_ * NCp[t]], F16, isOutput=False)
        P[f'idx_{t}'] = nc.declare_dram_parameter(f'idx_{t}', [128, sched[t]['Ttot']], I32, isOutput=False)
        P[f'dl_{t}'] = nc.declare_dram_parameter(f'dl_{t}', [128, sched[t]['Ttot']], F32, isOutput=False)
        P[f'oldidx_{t}'] = nc.declare_dram_parameter(f'oldidx_{t}', [128, NCp[t] // W], I32, isOutput=False)
    P['wts_b'] = nc.declare_dram_parameter('wts_b', [128, sched['b']['Ttot']], F32, isOutput=False)
    out_dec = nc.declare_dram_parameter('out', [NCp['f'], 1], F32, isOutput=True)

    # internal DRAM: per-core shard tables + replicated/gathered full tables
    S = {}
    HT = {}
    for t in 'bjf':
        HT[(0, t)] = nc.dram_tensor(f'h0{t}', [M_ * NCp[t], H], F16)
    for (l, t) in AG_NEEDED:
        S[(l, t)] = nc.dram_tensor(f's{l}{t}', [NCp[t], H], F16)
        HT[(l, t)] = nc.dram_tensor(f'h{l}{t}', [M_ * NCp[t], H], F16, addr_space='Shared')
    S[(3, 'f')] = nc.dram_tensor('s3f', [NCp['f'], H], F16)

    RG = [list(range(M_))]

    with tile.TileContext(nc) as tc:
        cpool = tc.alloc_tile_pool(name='consts', bufs=1)
        idxpool = tc.alloc_tile_pool(name='idxres', bufs=1)
        xpool = tc.alloc_tile_pool(name='xin', bufs=4)
        mpool = tc.alloc_tile_pool(name='msgs', bufs=24)
        spool = tc.alloc_tile_pool(name='sel', bufs=12)
        apool = tc.alloc_tile_pool(name='aggsb', bufs=6)
        opool = tc.alloc_tile_pool(name='oldsb', bufs=4)
        rpool = tc.alloc_tile_pool(name='ressb', bufs=6)
        agg_ps = tc.alloc_tile_pool(name='aggps', bufs=3, space='PSUM')
        out_ps = tc.alloc_tile_pool(name='outps', bufs=3, space='PSUM')
        tr_ps = tc.alloc_tile_pool(name='trps', bufs=2, space='PSUM')

        ident = cpool.tile([128, 128], F16, tag='ident', name='ident')
        make_identity(nc, ident[:, :])
        iota_sb = cpool.tile([128, W], F16, tag='iota', name='iota')
        nc.sync.dma_start(out=iota_sb[:, :], in_=P['iota'][:, :])

        nw_slots = prep['WPK'].shape[0]
        wsb = cpool.tile([128, nw_slots * 128], F16, tag='wsb', name='wsb')
        nc.sync.dma_start(out=wsb[:, :], in_=P['wpk'][:, :, :].rearrange('w p h -> p w h'))

        def Wk(key):
            i = wkeys[key]
            return wsb[:, i * 128:(i + 1) * 128]

        nb = prep['BPK'].shape[1]
        bsb = cpool.tile([128, nb], F32, tag='bsb', name='bsb')
        nc.sync.dma_start(out=bsb[:, :], in_=P['bpk'][:, :])

        def Bk(key):
            i = bkeys[key]
            return bsb[:, i:i + 1]

        idx_sb, dl_sb, oidx_sb = {}, {}, {}
        for t in 'bjf':
            tt = sched[t]['Ttot']
            idx_sb[t] = idxpool.tile([128, tt], I32, tag=f'idx{t}', name=f'idx{t}')
            nc.sync.dma_start(out=idx_sb[t][:, :], in_=P[f'idx_{t}'][:, :])
            dl_sb[t] = idxpool.tile([128, tt], F32, tag=f'dl{t}', name=f'dl{t}')
            nc.sync.dma_start(out=dl_sb[t][:, :], in_=P[f'dl_{t}'][:, :])
            oidx_sb[t] = idxpool.tile([128, NCp[t] // W], I32, tag=f'oi{t}', name=f'oi{t}')
            nc.sync.dma_start(out=oidx_sb[t][:, :], in_=P[f'oldidx_{t}'][:, :])
        wts_sb = idxpool.tile([128, sched['b']['Ttot']], F32, tag='wtsb', name='wtsb')
        nc.sync.dma_start(out=wts_sb[:, :], in_=P['wts_b'][:, :])

        for rep in range(REPEAT):
            # ---- replicated encoder: full fp16 tables on every core ----
            for t in 'bjf':
                nin = x_in[t] + 1      # features + constant-1 bias column
                for g in range(M_ * NCp[t] // W):
                    xtile = xpool.tile([128, W], F16, tag='xt', name='xt')
                    nc.sync.dma_start(out=xtile[:nin, :],
                                      in_=P[f'xt_{t}'][:, g * W:(g + 1) * W])
                    ps = out_ps.tile([128, W], F32, tag='outp', name='encp')
                    for k in range(W // 128):
                        nc.tensor.matmul(out=ps[:, k * 128:(k + 1) * 128],
                                         lhsT=xtile[:nin, k * 128:(k + 1) * 128],
                                         rhs=Wk(('enc', t))[:nin, :],
                                         start=True, stop=True)
                    orow = rpool.tile([128, W], F16, tag='orow', name='encw')
                    nc.scalar.activation(orow[:, :], ps[:, :],
                                         mybir.ActivationFunctionType.Relu,
                                         bias=0.0, scale=1.0)
                    nc.sync.dma_start(
                        out=HT[(0, t)][g * W:(g + 1) * W, :].rearrange('(p k) h -> p (k h)', k=4),
                        in_=orow[:, :])

            # ---- layers ----
            for l1 in range(1, L + 1):
                lw = l1 - 1
                for t in LAYER_TYPES[l1]:
                    ets = DST_ETS[t]
                    sc = sched[t]
                    for g in range(NCp[t] // W):
                        # old rows (this core's shard, interleaved layout)
                        old = opool.tile([128, W], F16, tag='old', name='old')
                        if l1 == 1:
                            nc.gpsimd.indirect_dma_start(
                                out=old[:, :], out_offset=None,
                                in_=HT[(0, t)][:, :],
                                in_offset=bass.IndirectOffsetOnAxis(
                                    ap=oidx_sb[t][:, g:g + 1], axis=0))
                        else:
                            nc.sync.dma_start(
                                out=old[:, :],
                                in_=S[(l1 - 1, t)][g * W:(g + 1) * W, :].rearrange('(p k) h -> p (k h)', k=4))
                        tp = tr_ps.tile([128, W], F16, tag='trp16', name='oldt')
                        for k in range(W // 128):
                            nc.tensor.transpose(out=tp[:, k * 128:(k + 1) * 128],
                                                in_=old[:, k * 128:(k + 1) * 128],
                                                identity=ident[:, :])
                        oldT = opool.tile([128, W], F16, tag='oldT', name='oldT')
                        nc.scalar.copy(oldT[:, :], tp[:, :])

                        aggs = []
                        for etn in ets:
                            Tn = int(sc['T'][etn][g])
                            if Tn == 0:
                                continue
                            t0 = sc['tile_base'][(etn, g)]
                            src_t = ETS[etn][0]
                            htab = HT[(0, src_t)] if l1 == 1 else HT[(l1 - 1, src_t)]
                            aps = agg_ps.tile([128, W], F32, tag='aggp', name='aggp')
                            for kk in range(Tn):
                                ti = t0 + kk
                                msgs = mpool.tile([128, H], F16, tag='msgs', name='msgs')
                                nc.gpsimd.indirect_dma_start(
                                    out=msgs[:, :], out_offset=None,
                                    in_=htab[:, :],
                                    in_offset=bass.IndirectOffsetOnAxis(
                                        ap=idx_sb[t][:, ti:ti + 1], axis=0))
                                sel = spool.tile([128, W], F16, tag='sel', name='sel')
                                if t == 'b':
                                    nc.vector.tensor_scalar(
                                        out=sel[:, :], in0=iota_sb[:, :],
                                        scalar1=dl_sb[t][:, ti:ti + 1],
                                        scalar2=wts_sb[:, ti:ti + 1],
                                        op0=mybir.AluOpType.is_equal,
                                        op1=mybir.AluOpType.mult)
                                else:
                                    nc.vector.tensor_scalar(
                                        out=sel[:, :], in0=iota_sb[:, :],
                                        scalar1=dl_sb[t][:, ti:ti + 1],
                                        scalar2=None,
                                        op0=mybir.AluOpType.is_equal)
                                nc.tensor.matmul(out=aps[:, :], lhsT=msgs[:, :],
                                                 rhs=sel[:, :],
                                                 start=(kk == 0), stop=(kk == Tn - 1))
                            asb = apool.tile([128, W], F16, tag='aggsb', name='aggsb')
                            nc.scalar.copy(asb[:, :], aps[:, :])
                            aggs.append((etn, asb))

                        ops = out_ps.tile([128, W], F32, tag='outp', name='outp')
                        nc.tensor.matmul(out=ops[:, :], lhsT=Wk(('root', lw, t)),
                                         rhs=oldT[:, :], start=True, stop=(not aggs))
                        for i, (etn, asb) in enumerate(aggs):
                            nc.tensor.matmul(out=ops[:, :], lhsT=Wk(('rel', lw, etn)),
                                             rhs=asb[:, :], start=False,
                                             stop=(i == len(aggs) - 1))

                        if t == 'b':
                            s0 = rpool.tile([128, W], F16, tag='res', name='s0')
                            nc.vector.tensor_scalar(
                                out=s0[:, :], in0=ops[:, :],
                                scalar1=Bk(('agg', lw, 'b')), scalar2=None,
                                op0=mybir.AluOpType.add)
                            ps2 = out_ps.tile([128, W], F32, tag='outp', name='mlp1p')
                            nc.tensor.matmul(out=ps2[:, :], lhsT=Wk(('mlp1',)),
                                             rhs=s0[:, :], start=True, stop=True)
                            t1 = rpool.tile([128, W], F16, tag='res', name='t1')
                            nc.scalar.activation(t1[:, :], ps2[:, :],
                                                 mybir.ActivationFunctionType.Relu,
                                                 bias=Bk(('mlp1',)), scale=1.0)
                            ps3 = out_ps.tile([128, W], F32, tag='outp', name='mlp2p')
                            nc.tensor.matmul(out=ps3[:, :], lhsT=Wk(('mlp2',)),
                                             rhs=t1[:, :], start=True, stop=True)
                            s = rpool.tile([128, W], F16, tag='res', name='sres')
                            nc.vector.tensor_scalar(
                                out=s[:, :], in0=ps3[:, :],
                                scalar1=Bk(('mlp2',)), scalar2=None,
                                op0=mybir.AluOpType.add)
                        else:
                            s = rpool.tile([128, W], F16, tag='res', name='sres')
                            nc.scalar.activation(s[:, :], ops[:, :],
                                                 mybir.ActivationFunctionType.Relu,
                                                 bias=Bk(('agg', lw, t)), scale=1.0)

                        # residual in transposed space
                        sr = rpool.tile([128, W], F16, tag='sr', name='sr')
                        nc.vector.tensor_add(sr[:, :], s[:, :], oldT[:, :])

                        if l1 == L:
                            dps = out_ps.tile([128, W], F32, tag='outp', name='decp')
                            for k in range(W // 128):
                                nc.tensor.matmul(out=dps[:, k:k + 1],
                                                 lhsT=sr[:, k * 128:(k + 1) * 128],
                                                 rhs=Wk(('dec',))[:, 0:1],
                                                 start=True, stop=True)
                            dsb = rpool.tile([128, W // 128], F32, tag='dsb', name='dsb')
                            nc.scalar.activation(dsb[:, :], dps[:, :W // 128],
                                                 mybir.ActivationFunctionType.Copy,
                                                 bias=prep['dec_b'], scale=1.0)
                            nc.sync.dma_start(
                                out=out_dec[g * W:(g + 1) * W, :].rearrange('(k p) o -> p k o', p=128),
                                in_=dsb[:, :])
                        else:
                            tp2 = tr_ps.tile([128, W], F16, tag='trp16', name='newt')
                            for k in range(W // 128):
                                nc.tensor.transpose(out=tp2[:, k * 128:(k + 1) * 128],
                                                    in_=sr[:, k * 128:(k + 1) * 128],
                                                    identity=ident[:, :])
                            orow = rpool.tile([128, W], F16, tag='orow', name='orow')
                            nc.scalar.copy(orow[:, :], tp2[:, :])
                            nc.sync.dma_start(
                                out=S[(l1, t)][g * W:(g + 1) * W, :].rearrange('(p k) h -> p (k h)', k=4),
                                in_=orow[:, :])

                    if (l1, t) in AG_NEEDED:
                        nc.gpsimd.collective_compute(
                            'AllGather', mybir.AluOpType.bypass, replica_groups=RG,
                            ins=[S[(l1, t)][:, :]], outs=[HT[(l1, t)][:, :]])

        for p in (tr_ps, out_ps, agg_ps, rpool, opool, apool, spool, mpool,
                  xpool, idxpool, cpool):
            p.release()

    nc.compile()
    return nc


# ------------------------------------------------------------------ driver ----
def postprocess(res, prep):
    cfg = prep['cfg']
    M_ = cfg['M']
    NF = cfg['N']['f']
    outp = np.concatenate([np.asarray(res[c]['out']) for c in range(M_)], 0)
    out = np.empty((NF, 1), np.float32)
    out[:, 0] = outp[prep['n_log_of_old']['f'], 0]
    return out


def kernel(**inputs):
    prep = preprocess(inputs, FULL_CFG)
    in_maps = make_in_maps(prep)
    nc = build_program(prep)
    res = run_bass_kernel_spmd(nc, in_maps, list(range(FULL_CFG['M']))).results
    return postprocess(res, prep)
